# revision 17
# baseline (speedup 1.0000x reference)
"""Trainium2 Bass kernel for the fused soft-logic-gate layer.

Reference computation:
    pa = softmax(wa, axis=1); pb = softmax(wb, axis=1); pt = softmax(wt, axis=0)
    A = pa @ x; B = pb @ x
    out = sum_g pt[g,:,None] * gate_g(A, B)        (16 soft logic gates)

Every gate is affine in {1, A, B, A*B}, so the 16-gate table collapses to
    out = c0 + cA*A + cB*B + cAB*(A*B)
with four per-row coefficient vectors derived from pt.  Folding the softmax
denominators of wa/wb into those coefficients lets the matmuls run on the raw
exp() weights, and factoring
    out = (A + u) * (cAB*B + cA) + w,   u = cB/cAB,  w = c0 - cA*u
leaves one ACT op + two DVE ops per tile.  The device work is two
[256,256]x[256,b] float32r (TF32) matmuls plus that elementwise pass —
memory-bound on streaming x in and out once.

Sharding: batch axis of x split evenly across 8 NeuronCores (data parallel),
weights replicated.
"""

import os
import sys

for _p in ("/opt/trn_rl_repo",):
    if _p not in sys.path and os.path.isdir(_p):
        sys.path.insert(0, _p)

import numpy as np

SIZE = 256
PREV = 256
BATCH = 32768
N_CORES = 8
BSH = BATCH // N_CORES  # per-core batch shard
CH = 1024               # epilogue chunk width (2 PSUM banks)
NCH = BSH // CH
P = 128

# constants blob layout (f32, [128, 390]):
#   [:, 0:128]     identity
#   [:, 128]       ones column
#   [:16, 129:134] sign matrix [16, 5] (cols: sum, c0, cA, cB, cAB)
#   [:16, 134:390] wt [16, 256]
BLOB_W = 390

_CACHE = {}


def _sign_matrix() -> np.ndarray:
    """[16,5] f32 columns: [colsum, c0, cA, cB, cAB] — gate-table
    coefficients of {1, A, B, A*B} preceded by the softmax denominator."""
    S = np.zeros((16, 5), dtype=np.float32)
    S[:, 0] = 1.0
    S[8:16, 1] = 1.0
    for g in (2, 3, 6, 7):
        S[g, 2] += 1.0
    for g in (8, 9, 12, 13):
        S[g, 2] -= 1.0
    for g in (4, 5, 6, 7):
        S[g, 3] += 1.0
    for g in (8, 9, 10, 11):
        S[g, 3] -= 1.0
    for g, v in {1: 1, 2: -1, 4: -1, 6: -2, 7: -1, 8: 1, 9: 2, 11: 1, 13: 1, 14: -1}.items():
        S[g, 4] = v
    return S


def _build_bass():
    import concourse.bacc as bacc
    import concourse.tile as tile
    import concourse.mybir as mybir

    f32 = mybir.dt.float32
    f32r = mybir.dt.float32r
    Act = mybir.ActivationFunctionType
    Alu = mybir.AluOpType

    nc = bacc.Bacc(trn_type="TRN2", target_bir_lowering=False, debug=False,
                   num_devices=N_CORES)

    xs_d = nc.dram_tensor("xs", [PREV, BSH], f32r, kind="ExternalInput").ap()
    wa_d = nc.dram_tensor("wa", [SIZE, PREV], f32, kind="ExternalInput").ap()
    wb_d = nc.dram_tensor("wb", [SIZE, PREV], f32, kind="ExternalInput").ap()
    bl_d = nc.dram_tensor("blob", [P, BLOB_W], f32, kind="ExternalInput").ap()
    out_d = nc.dram_tensor("out", [SIZE, BSH], f32, kind="ExternalOutput").ap()

    # [p, k/m, b] views for single-DMA transfers
    xs_v = xs_d.rearrange("(k p) b -> p k b", p=P)
    wa_v = wa_d.rearrange("(m p) c -> p m c", p=P)
    wb_v = wb_d.rearrange("(m p) c -> p m c", p=P)

    with tile.TileContext(nc, pool_alloc_mode="queue") as tc:
        with tc.tile_pool(name="consts", bufs=1) as consts, \
             tc.tile_pool(name="weights", bufs=1) as weights, \
             tc.tile_pool(name="coefs", bufs=1) as coefs, \
             tc.tile_pool(name="xp", bufs=4) as xp:

            blob = consts.tile([P, BLOB_W], f32)
            nc.sync.dma_start(out=blob[:], in_=bl_d[:])
            ident = blob[:, 0:128]
            smat = blob[:16, 129:134]
            wts = blob[:16, 134:390]

            # tiny early Exp forces the ACT table load off the critical path
            dummy = consts.tile([1, 1], f32)
            nc.scalar.activation(out=dummy[:], in_=blob[0:1, 128:129], func=Act.Exp)

            wa_sb = consts.tile([P, 2, PREV], f32)
            nc.sync.dma_start(out=wa_sb[:], in_=wa_v[:])
            wb_sb = consts.tile([P, 2, PREV], f32)
            nc.sync.dma_start(out=wb_sb[:], in_=wb_v[:])

            # prefetch the first x chunks
            xtiles = []
            for n in range(2):
                xt = xp.tile([P, 2, CH], f32r, tag="x", name=f"x{n}")
                nc.sync.dma_start(out=xt[:], in_=xs_v[:, :, n * CH:(n + 1) * CH])
                xtiles.append(xt)

            # Transposed exp(weights), float32r, live for the whole kernel:
            # eaT[p] is [128(prev-block p), 256(size)].
            eaT = [weights.tile([P, SIZE], f32r, tag=f"eaT{p}", name=f"eaT{p}") for p in range(2)]
            ebT = [weights.tile([P, SIZE], f32r, tag=f"ebT{p}", name=f"ebT{p}") for p in range(2)]

            # [128,2] coefficient tiles (m as free dim):
            cA2 = coefs.tile([P, 2], f32, tag="cA2")
            cAB2 = coefs.tile([P, 2], f32, tag="cAB2")
            cU2 = coefs.tile([P, 2], f32, tag="cU2")
            cW2 = coefs.tile([P, 2], f32, tag="cW2")

            # ---- weight preprocessing ----
            with tc.tile_pool(name="prep", bufs=2) as prep, \
                 tc.tile_pool(name="prep_ps", bufs=3, space="PSUM") as prep_ps, \
                 tc.tile_pool(name="coef_ps", bufs=1, space="PSUM") as coef_ps:

                # pt-coefficient path first: it only needs the blob
                ept = prep.tile([16, SIZE], f32, tag="ept")
                nc.scalar.activation(out=ept[:], in_=wts, func=Act.Exp)
                cps = coef_ps.tile([P, 10], f32, tag="cps")
                for m in range(2):
                    nc.tensor.matmul(cps[:, m * 5:(m + 1) * 5],
                                     ept[:, m * P:(m + 1) * P], smat,
                                     start=True, stop=True)
                cpsv = cps[:].rearrange("p (m c) -> p c m", m=2)
                rpt2 = prep.tile([P, 2], f32, tag="rpt2")
                nc.vector.reciprocal(out=rpt2[:], in_=cpsv[:, 0, :])
                rcabn = prep.tile([P, 2], f32, tag="rcabn")
                nc.vector.reciprocal(out=rcabn[:], in_=cpsv[:, 4, :])

                # exp in natural layout (one wide ACT op per weight), row sums
                # on DVE, then PE-transpose each block and copy out as float32r
                rsa = prep.tile([P, 2], f32, tag="rsa")
                rsb = prep.tile([P, 2], f32, tag="rsb")
                for w_sb, eT, rs, nm in ((wa_sb, eaT, rsa, "a"), (wb_sb, ebT, rsb, "b")):
                    e_nat = prep.tile([P, 2, PREV], f32, tag=f"e{nm}", name=f"e{nm}")
                    nc.scalar.activation(out=e_nat[:], in_=w_sb[:], func=Act.Exp)
                    for m in range(2):
                        nc.vector.tensor_reduce(out=rs[:, m:m + 1], in_=e_nat[:, m, :],
                                                axis=mybir.AxisListType.X, op=Alu.add)
                    for m in range(2):
                        for p in range(2):
                            tp = prep_ps.tile([P, P], f32, tag="tps", name=f"tp{nm}{m}{p}")
                            nc.tensor.transpose(tp[:], e_nat[:, m, p * P:(p + 1) * P], ident)
                            nc.scalar.copy(out=eT[p][:, m * P:(m + 1) * P], in_=tp[:])

                ra2 = prep.tile([P, 2], f32, tag="ra2")
                nc.vector.reciprocal(out=ra2[:], in_=rsa[:])
                rb2 = prep.tile([P, 2], f32, tag="rb2")
                nc.vector.reciprocal(out=rb2[:], in_=rsb[:])

                # batched [128,2] coefficient chain:
                h2 = prep.tile([P, 2], f32, tag="h2")
                nc.vector.tensor_tensor(out=h2[:], in0=rpt2[:], in1=ra2[:], op=Alu.mult)
                nc.vector.tensor_tensor(out=cA2[:], in0=cpsv[:, 2, :], in1=h2[:], op=Alu.mult)
                g2 = prep.tile([P, 2], f32, tag="g2")
                nc.vector.tensor_tensor(out=g2[:], in0=h2[:], in1=rb2[:], op=Alu.mult)
                nc.vector.tensor_tensor(out=cAB2[:], in0=cpsv[:, 4, :], in1=g2[:], op=Alu.mult)

                # u = cBn * sa / cABn ;  w = c0n*rpt - cA*u
                u2a = prep.tile([P, 2], f32, tag="u2a")
                nc.vector.tensor_tensor(out=u2a[:], in0=cpsv[:, 3, :], in1=rcabn[:], op=Alu.mult)
                nc.vector.tensor_tensor(out=cU2[:], in0=u2a[:], in1=rsa[:], op=Alu.mult)
                c02 = prep.tile([P, 2], f32, tag="c02")
                nc.vector.tensor_tensor(out=c02[:], in0=cpsv[:, 1, :], in1=rpt2[:], op=Alu.mult)
                t2 = prep.tile([P, 2], f32, tag="t2")
                nc.vector.tensor_tensor(out=t2[:], in0=cA2[:], in1=cU2[:], op=Alu.mult)
                nc.vector.tensor_tensor(out=cW2[:], in0=c02[:], in1=t2[:], op=Alu.subtract)

            # ---- main loop ----
            with tc.tile_pool(name="ep", bufs=3) as ep, \
                 tc.tile_pool(name="mm_ps", bufs=2, space="PSUM") as mm_ps:
                for n in range(NCH):
                    if n + 2 < NCH:
                        xt = xp.tile([P, 2, CH], f32r, tag="x", name=f"x{n+2}")
                        nc.sync.dma_start(out=xt[:], in_=xs_v[:, :, (n + 2) * CH:(n + 3) * CH])
                        xtiles.append(xt)
                    xk = xtiles[n]
                    for m in range(2):
                        a_ps = mm_ps.tile([P, CH], f32, tag="A", name=f"A{n}{m}")
                        b_ps = mm_ps.tile([P, CH], f32, tag="B", name=f"B{n}{m}")
                        for ps_t, eT in ((a_ps, eaT), (b_ps, ebT)):
                            for k in range(2):
                                for s in range(CH // 512):
                                    sl = slice(s * 512, (s + 1) * 512)
                                    nc.tensor.matmul(ps_t[:, sl],
                                                     eT[k][:, m * P:(m + 1) * P],
                                                     xk[:, k, sl],
                                                     start=(k == 0), stop=(k == 1))
                        # out = (A + u) * (cAB*B + cA) + w
                        s_sb = ep.tile([P, CH], f32, tag="s", name=f"s{n}{m}")
                        nc.scalar.activation(out=s_sb[:], in_=b_ps[:], func=Act.Identity,
                                             scale=cAB2[:, m:m + 1], bias=cA2[:, m:m + 1])
                        p_sb = ep.tile([P, CH], f32, tag="p", name=f"p{n}{m}")
                        nc.vector.scalar_tensor_tensor(out=p_sb[:], in0=a_ps[:],
                                                       scalar=cU2[:, m:m + 1], in1=s_sb[:],
                                                       op0=Alu.add, op1=Alu.mult)
                        o_sb = ep.tile([P, CH], f32, tag="o", name=f"o{n}{m}")
                        nc.vector.tensor_scalar_add(o_sb[:], p_sb[:], cW2[:, m:m + 1])
                        if n == NCH - 1 and m == 1:
                            hw = CH // 2
                            for h in range(2):
                                nc.sync.dma_start(
                                    out=out_d[m * P:(m + 1) * P,
                                              n * CH + h * hw:n * CH + (h + 1) * hw],
                                    in_=o_sb[:, h * hw:(h + 1) * hw])
                        else:
                            nc.sync.dma_start(out=out_d[m * P:(m + 1) * P, n * CH:(n + 1) * CH],
                                              in_=o_sb[:])

    nc.compile()
    return nc


def _get_nc():
    if "nc" not in _CACHE:
        _CACHE["nc"] = _build_bass()
    return _CACHE["nc"]


def _make_blob(wt: np.ndarray) -> np.ndarray:
    blob = np.zeros((P, BLOB_W), dtype=np.float32)
    blob[:, 0:128] = np.eye(P, dtype=np.float32)
    blob[:, 128] = 1.0
    blob[:16, 129:134] = _sign_matrix()
    blob[:16, 134:390] = wt
    return blob


def _run(x, wa, wb, wt, trace=False, **spmd_kwargs):
    from concourse import bass_utils

    nc = _get_nc()
    x = np.ascontiguousarray(np.asarray(x, dtype=np.float32))
    wa = np.ascontiguousarray(np.asarray(wa, dtype=np.float32))
    wb = np.ascontiguousarray(np.asarray(wb, dtype=np.float32))
    wt = np.ascontiguousarray(np.asarray(wt, dtype=np.float32))
    blob = _make_blob(wt)

    in_maps = []
    for c in range(N_CORES):
        in_maps.append({
            "xs": np.ascontiguousarray(x[:, c * BSH:(c + 1) * BSH]),
            "wa": wa, "wb": wb, "blob": blob,
        })
    res = bass_utils.run_bass_kernel_spmd(nc, in_maps, core_ids=list(range(N_CORES)),
                                          trace=trace, **spmd_kwargs)
    out = np.concatenate([res.results[c]["out"] for c in range(N_CORES)], axis=1)
    return out, res


def kernel(x, wa, wb, wt):
    out, _ = _run(x, wa, wb, wt, trace=False)
    return out


# revision 18
# speedup vs baseline: 1.0207x; 1.0207x over previous
"""Trainium2 Bass kernel for the fused soft-logic-gate layer.

Reference computation:
    pa = softmax(wa, axis=1); pb = softmax(wb, axis=1); pt = softmax(wt, axis=0)
    A = pa @ x; B = pb @ x
    out = sum_g pt[g,:,None] * gate_g(A, B)        (16 soft logic gates)

Every gate is affine in {1, A, B, A*B}, so the 16-gate table collapses to
    out = c0 + cA*A + cB*B + cAB*(A*B)
with four per-row coefficient vectors derived from pt.  Folding the softmax
denominators of wa/wb into those coefficients lets the matmuls run on the raw
exp() weights, and factoring
    out = (A + u) * (cAB*B + cA) + w,   u = cB/cAB,  w = c0 - cA*u
leaves one ACT op + two DVE ops per tile.  The device work is two
[256,256]x[256,b] float32r (TF32) matmuls plus that elementwise pass —
memory-bound on streaming x in and out once.

Sharding: batch axis of x split evenly across 8 NeuronCores (data parallel),
weights replicated.
"""

import os
import sys

for _p in ("/opt/trn_rl_repo",):
    if _p not in sys.path and os.path.isdir(_p):
        sys.path.insert(0, _p)

import numpy as np

SIZE = 256
PREV = 256
BATCH = 32768
N_CORES = 8
BSH = BATCH // N_CORES  # per-core batch shard
CH = 1024               # epilogue chunk width (2 PSUM banks)
NCH = BSH // CH
P = 128

# constants blob layout (f32, [128, 390]):
#   [:, 0:128]     identity
#   [:, 128]       ones column
#   [:16, 129:134] sign matrix [16, 5] (cols: sum, c0, cA, cB, cAB)
#   [:16, 134:390] wt [16, 256]
BLOB_W = 390

_CACHE = {}


def _sign_matrix() -> np.ndarray:
    """[16,5] f32 columns: [colsum, c0, cA, cB, cAB] — gate-table
    coefficients of {1, A, B, A*B} preceded by the softmax denominator."""
    S = np.zeros((16, 5), dtype=np.float32)
    S[:, 0] = 1.0
    S[8:16, 1] = 1.0
    for g in (2, 3, 6, 7):
        S[g, 2] += 1.0
    for g in (8, 9, 12, 13):
        S[g, 2] -= 1.0
    for g in (4, 5, 6, 7):
        S[g, 3] += 1.0
    for g in (8, 9, 10, 11):
        S[g, 3] -= 1.0
    for g, v in {1: 1, 2: -1, 4: -1, 6: -2, 7: -1, 8: 1, 9: 2, 11: 1, 13: 1, 14: -1}.items():
        S[g, 4] = v
    return S


def _build_bass():
    import concourse.bacc as bacc
    import concourse.tile as tile
    import concourse.mybir as mybir

    f32 = mybir.dt.float32
    f32r = mybir.dt.float32r
    Act = mybir.ActivationFunctionType
    Alu = mybir.AluOpType

    nc = bacc.Bacc(trn_type="TRN2", target_bir_lowering=False, debug=False,
                   num_devices=N_CORES)

    xs_d = nc.dram_tensor("xs", [PREV, BSH], f32r, kind="ExternalInput").ap()
    wa_d = nc.dram_tensor("wa", [SIZE, PREV], f32, kind="ExternalInput").ap()
    wb_d = nc.dram_tensor("wb", [SIZE, PREV], f32, kind="ExternalInput").ap()
    bl_d = nc.dram_tensor("blob", [P, BLOB_W], f32, kind="ExternalInput").ap()
    out_d = nc.dram_tensor("out", [SIZE, BSH], f32, kind="ExternalOutput").ap()

    # [p, k/m, b] views for single-DMA transfers
    xs_v = xs_d.rearrange("(k p) b -> p k b", p=P)
    wa_v = wa_d.rearrange("(m p) c -> p m c", p=P)
    wb_v = wb_d.rearrange("(m p) c -> p m c", p=P)

    with tile.TileContext(nc) as tc:
        with tc.tile_pool(name="consts", bufs=1) as consts, \
             tc.tile_pool(name="weights", bufs=1) as weights, \
             tc.tile_pool(name="coefs", bufs=1) as coefs, \
             tc.tile_pool(name="xp", bufs=4) as xp:

            blob = consts.tile([P, BLOB_W], f32)
            nc.sync.dma_start(out=blob[:], in_=bl_d[:])
            ident = blob[:, 0:128]
            smat = blob[:16, 129:134]
            wts = blob[:16, 134:390]

            # tiny early Exp forces the ACT table load off the critical path
            dummy = consts.tile([1, 1], f32)
            nc.scalar.activation(out=dummy[:], in_=blob[0:1, 128:129], func=Act.Exp)

            wa_sb = consts.tile([P, 2, PREV], f32)
            nc.sync.dma_start(out=wa_sb[:], in_=wa_v[:])
            wb_sb = consts.tile([P, 2, PREV], f32)
            nc.sync.dma_start(out=wb_sb[:], in_=wb_v[:])

            # prefetch the first x chunks
            xtiles = []
            for n in range(2):
                xt = xp.tile([P, 2, CH], f32r, tag="x", name=f"x{n}")
                nc.sync.dma_start(out=xt[:], in_=xs_v[:, :, n * CH:(n + 1) * CH])
                xtiles.append(xt)

            # Transposed exp(weights), float32r, live for the whole kernel:
            # eaT[p] is [128(prev-block p), 256(size)].
            eaT = [weights.tile([P, SIZE], f32r, tag=f"eaT{p}", name=f"eaT{p}") for p in range(2)]
            ebT = [weights.tile([P, SIZE], f32r, tag=f"ebT{p}", name=f"ebT{p}") for p in range(2)]

            # [128,2] coefficient tiles (m as free dim):
            cA2 = coefs.tile([P, 2], f32, tag="cA2")
            cAB2 = coefs.tile([P, 2], f32, tag="cAB2")
            cU2 = coefs.tile([P, 2], f32, tag="cU2")
            cW2 = coefs.tile([P, 2], f32, tag="cW2")

            # ---- weight preprocessing ----
            with tc.tile_pool(name="prep", bufs=2) as prep, \
                 tc.tile_pool(name="prep_ps", bufs=3, space="PSUM") as prep_ps, \
                 tc.tile_pool(name="coef_ps", bufs=1, space="PSUM") as coef_ps:

                # pt-coefficient path first: it only needs the blob
                ept = prep.tile([16, SIZE], f32, tag="ept")
                nc.scalar.activation(out=ept[:], in_=wts, func=Act.Exp)
                cps = coef_ps.tile([P, 10], f32, tag="cps")
                for m in range(2):
                    nc.tensor.matmul(cps[:, m * 5:(m + 1) * 5],
                                     ept[:, m * P:(m + 1) * P], smat,
                                     start=True, stop=True)
                cpsv = cps[:].rearrange("p (m c) -> p c m", m=2)
                rpt2 = prep.tile([P, 2], f32, tag="rpt2")
                nc.vector.reciprocal(out=rpt2[:], in_=cpsv[:, 0, :])
                rcabn = prep.tile([P, 2], f32, tag="rcabn")
                nc.vector.reciprocal(out=rcabn[:], in_=cpsv[:, 4, :])

                # exp in natural layout (one wide ACT op per weight), row sums
                # on DVE, then PE-transpose each block and copy out as float32r
                rsa = prep.tile([P, 2], f32, tag="rsa")
                rsb = prep.tile([P, 2], f32, tag="rsb")
                for w_sb, eT, rs, nm in ((wa_sb, eaT, rsa, "a"), (wb_sb, ebT, rsb, "b")):
                    e_nat = prep.tile([P, 2, PREV], f32, tag=f"e{nm}", name=f"e{nm}")
                    nc.scalar.activation(out=e_nat[:], in_=w_sb[:], func=Act.Exp)
                    for m in range(2):
                        nc.vector.tensor_reduce(out=rs[:, m:m + 1], in_=e_nat[:, m, :],
                                                axis=mybir.AxisListType.X, op=Alu.add)
                    for m in range(2):
                        for p in range(2):
                            tp = prep_ps.tile([P, P], f32, tag="tps", name=f"tp{nm}{m}{p}")
                            nc.tensor.transpose(tp[:], e_nat[:, m, p * P:(p + 1) * P], ident)
                            nc.scalar.copy(out=eT[p][:, m * P:(m + 1) * P], in_=tp[:])

                ra2 = prep.tile([P, 2], f32, tag="ra2")
                nc.vector.reciprocal(out=ra2[:], in_=rsa[:])
                rb2 = prep.tile([P, 2], f32, tag="rb2")
                nc.vector.reciprocal(out=rb2[:], in_=rsb[:])

                # batched [128,2] coefficient chain:
                h2 = prep.tile([P, 2], f32, tag="h2")
                nc.vector.tensor_tensor(out=h2[:], in0=rpt2[:], in1=ra2[:], op=Alu.mult)
                nc.vector.tensor_tensor(out=cA2[:], in0=cpsv[:, 2, :], in1=h2[:], op=Alu.mult)
                g2 = prep.tile([P, 2], f32, tag="g2")
                nc.vector.tensor_tensor(out=g2[:], in0=h2[:], in1=rb2[:], op=Alu.mult)
                nc.vector.tensor_tensor(out=cAB2[:], in0=cpsv[:, 4, :], in1=g2[:], op=Alu.mult)

                # u = cBn * sa / cABn ;  w = c0n*rpt - cA*u
                u2a = prep.tile([P, 2], f32, tag="u2a")
                nc.vector.tensor_tensor(out=u2a[:], in0=cpsv[:, 3, :], in1=rcabn[:], op=Alu.mult)
                nc.vector.tensor_tensor(out=cU2[:], in0=u2a[:], in1=rsa[:], op=Alu.mult)
                c02 = prep.tile([P, 2], f32, tag="c02")
                nc.vector.tensor_tensor(out=c02[:], in0=cpsv[:, 1, :], in1=rpt2[:], op=Alu.mult)
                t2 = prep.tile([P, 2], f32, tag="t2")
                nc.vector.tensor_tensor(out=t2[:], in0=cA2[:], in1=cU2[:], op=Alu.mult)
                nc.vector.tensor_tensor(out=cW2[:], in0=c02[:], in1=t2[:], op=Alu.subtract)

            # ---- main loop ----
            with tc.tile_pool(name="ep", bufs=3) as ep, \
                 tc.tile_pool(name="mm_ps", bufs=2, space="PSUM") as mm_ps:
                for n in range(NCH):
                    if n + 2 < NCH:
                        xt = xp.tile([P, 2, CH], f32r, tag="x", name=f"x{n+2}")
                        nc.sync.dma_start(out=xt[:], in_=xs_v[:, :, (n + 2) * CH:(n + 3) * CH])
                        xtiles.append(xt)
                    xk = xtiles[n]
                    for m in range(2):
                        a_ps = mm_ps.tile([P, CH], f32, tag="A", name=f"A{n}{m}")
                        b_ps = mm_ps.tile([P, CH], f32, tag="B", name=f"B{n}{m}")
                        for ps_t, eT in ((a_ps, eaT), (b_ps, ebT)):
                            for k in range(2):
                                for s in range(CH // 512):
                                    sl = slice(s * 512, (s + 1) * 512)
                                    nc.tensor.matmul(ps_t[:, sl],
                                                     eT[k][:, m * P:(m + 1) * P],
                                                     xk[:, k, sl],
                                                     start=(k == 0), stop=(k == 1))
                        # out = (A + u) * (cAB*B + cA) + w
                        s_sb = ep.tile([P, CH], f32, tag="s", name=f"s{n}{m}")
                        nc.scalar.activation(out=s_sb[:], in_=b_ps[:], func=Act.Identity,
                                             scale=cAB2[:, m:m + 1], bias=cA2[:, m:m + 1])
                        p_sb = ep.tile([P, CH], f32, tag="p", name=f"p{n}{m}")
                        nc.vector.scalar_tensor_tensor(out=p_sb[:], in0=a_ps[:],
                                                       scalar=cU2[:, m:m + 1], in1=s_sb[:],
                                                       op0=Alu.add, op1=Alu.mult)
                        o_sb = ep.tile([P, CH], f32, tag="o", name=f"o{n}{m}")
                        nc.vector.tensor_scalar_add(o_sb[:], p_sb[:], cW2[:, m:m + 1])
                        if n == NCH - 1 and m == 1:
                            hw = CH // 2
                            for h in range(2):
                                nc.sync.dma_start(
                                    out=out_d[m * P:(m + 1) * P,
                                              n * CH + h * hw:n * CH + (h + 1) * hw],
                                    in_=o_sb[:, h * hw:(h + 1) * hw])
                        else:
                            nc.sync.dma_start(out=out_d[m * P:(m + 1) * P, n * CH:(n + 1) * CH],
                                              in_=o_sb[:])

    nc.compile()
    return nc


def _get_nc():
    if "nc" not in _CACHE:
        _CACHE["nc"] = _build_bass()
    return _CACHE["nc"]


def _make_blob(wt: np.ndarray) -> np.ndarray:
    blob = np.zeros((P, BLOB_W), dtype=np.float32)
    blob[:, 0:128] = np.eye(P, dtype=np.float32)
    blob[:, 128] = 1.0
    blob[:16, 129:134] = _sign_matrix()
    blob[:16, 134:390] = wt
    return blob


def _run(x, wa, wb, wt, trace=False, **spmd_kwargs):
    from concourse import bass_utils

    nc = _get_nc()
    x = np.ascontiguousarray(np.asarray(x, dtype=np.float32))
    wa = np.ascontiguousarray(np.asarray(wa, dtype=np.float32))
    wb = np.ascontiguousarray(np.asarray(wb, dtype=np.float32))
    wt = np.ascontiguousarray(np.asarray(wt, dtype=np.float32))
    blob = _make_blob(wt)

    in_maps = []
    for c in range(N_CORES):
        in_maps.append({
            "xs": np.ascontiguousarray(x[:, c * BSH:(c + 1) * BSH]),
            "wa": wa, "wb": wb, "blob": blob,
        })
    res = bass_utils.run_bass_kernel_spmd(nc, in_maps, core_ids=list(range(N_CORES)),
                                          trace=trace, **spmd_kwargs)
    out = np.concatenate([res.results[c]["out"] for c in range(N_CORES)], axis=1)
    return out, res


def kernel(x, wa, wb, wt):
    out, _ = _run(x, wa, wb, wt, trace=False)
    return out
